# revision 50
# baseline (speedup 1.0000x reference)
"""Adaptive-attention LSTM (B=32,T=64,HID=512,K=L=49,VOCAB=10000) on 8 TRN2 cores.

Strategy (v3)
-------------
Data-parallel over batch: each core gets B_local=4 sequences, feature-on-
partition / (t,b)-on-free layout throughout.

  loads:   one coalesced strided DMA descriptor per tensor (descriptor issue
           costs ~0.7us of engine time), on the SP+SWDGE queues only so the
           ACT engine stays free; Wmlp preloaded whole into SBUF (80KB/
           partition) so the MLP never touches HBM.
  phase A: x_gates = Wih@x + bias for t<32 up front; the t>=32 half plus the
           xs/Vk projections issue one-per-step inside phase B's PE gaps.
  phase B: 64-step serial LSTM. Gate blocks ordered [g,i,f,o] with one PSUM
           tile per block so each block's +xg add / activation starts as soon
           as that block's matmuls retire (the burst is LDW-bound; the chain
           hides under its tail). h written directly as bf16.
  phase C: attention + sentinel, all in tanh/exp form (the sentinel sigmoid
           is computed as (tanh(x/2)+1)/2 with the 0.5's folded into Ws and
           the beta broadcast), so phases B and C each use a single ACT
           table set and the scheduler may interleave phase C and the MLP
           into idle engine slots without table thrash. Exact folded softmax
           u = (e_how*s + e_z@V)/D + h with D = sum(e_z)+e_how.
  MLP:     y = u @ Wmlp.T, PSUM->bf16 casts alternating DVE/ACT, stores
           batched 5 vocab-chunks per descriptor; bmlp added on host.
"""

import os
import sys
import types

for _p in ("/opt/pypackages", "/opt/trn_rl_repo"):
    if _p not in sys.path and os.path.isdir(_p):
        sys.path.insert(0, _p)


def _install_ntff_shim():
    """antenv.axon_hooks is missing in the trimmed repo; provide it so
    run_bass_kernel_spmd(trace=True) can reach the NTFF profile hook."""
    if "antenv.axon_hooks" in sys.modules:
        return
    try:
        from trn_agent_boot.trn_boot import _ntff_profile_via_ctypes

        hook = _ntff_profile_via_ctypes("/opt/axon/libaxon_pjrt.so")
    except Exception:
        hook = None
    m = types.ModuleType("antenv.axon_hooks")
    m.get_axon_ntff_profile_hook = lambda: hook
    m.set_axon_ntff_profile_hook = lambda h: None
    sys.modules["antenv.axon_hooks"] = m


_install_ntff_shim()


def _enable_ldw_opt():
    """compile flags hardcode --enable-ldw-opt=false; flip it so LDWEIGHTS can
    use the fast-weight-load path."""
    import concourse.bass_utils as bu

    if os.environ.get("BASS_LDW_OPT", "1") != "1":
        return
    if getattr(bu, "_ldw_patched", False):
        return
    orig = bu.run_command

    def patched(argv, **kw):
        argv = [a.replace("--enable-ldw-opt=false", "--enable-ldw-opt=true")
                if isinstance(a, str) else a for a in argv]
        return orig(argv, **kw)

    bu.run_command = patched
    bu._ldw_patched = True


_enable_ldw_opt()

import ml_dtypes
import numpy as np

import concourse.bass as bass
import concourse.mybir as mybir
import concourse.tile as tile
from concourse.bass_utils import run_bass_kernel_spmd
from concourse.tile import add_dep_helper
from concourse.vector_clock import ScopedClock


def _patch_tile_drain():
    """This walrus build allows a single sync-wait per CTRL instruction; Tile's
    tail drain attaches one wait per live semaphore. Spread them over a chain
    of SP nops (sequential on SP -> identical semantics)."""
    if getattr(tile.TileContext, "_drain_patched", False):
        return

    def _drain_and_barrier(self, tick_clock, wait_clock):
        nc = self.nc
        probe = nc.sync.nop(nofuse=True)
        wait_clock.add_sem_waits(
            probe.ins, ScopedClock({None: tick_clock.global_clock})
        )
        waits = list(probe.ins.sync_info.on_wait)
        if len(waits) > 1:
            probe.ins.sync_info.on_wait = waits[:1]
            for i in range(1, len(waits)):
                extra = nc.sync.nop(nofuse=True)
                if extra.ins.sync_info is None:
                    extra.ins.sync_info = mybir.SyncInfo(
                        on_wait=waits[i : i + 1], on_update=[]
                    )
                else:
                    extra.ins.sync_info.on_wait = waits[i : i + 1]
        nc.sync.drain()
        nc.all_engine_barrier()
        assert self.sems is not None
        popped = nc._tile_sem_poison_stack.pop()
        assert popped is self._sem_poison
        nc.clear_and_free_semaphores(list(self.sems.allocated().values()))
        nc.all_engine_barrier()

    tile.TileContext._drain_and_barrier = _drain_and_barrier
    tile.TileContext._drain_patched = True

    # General pass: the ISA here allows a single sync-wait per instruction.
    # Before lowering, split any instruction with N>1 waits into N-1 preceding
    # single-wait NOPs on the same engine (engine streams execute in order, so
    # semantics are identical).
    _orig_lower = tile.TileContext._lower_ordered_insts

    def _split_multi_waits(self, ordered):
        nc = self.nc
        # Sweep 1: fuse standalone InstLdweights back into their InstMatmult
        # (self-loading form, ldweights=True). The standalone form defeats the
        # codegen fast-weight-load path. Pair FIFO by PE stream order; merge
        # sync lists.
        fuse = os.environ.get("BASS_FUSE_LDW", "1") == "1"
        for insts in ordered.values():
            if not fuse:
                break
            fused_out = []
            pending = []  # (index in fused_out, ldw inst)
            for inst in insts:
                tn = type(inst).__name__
                if tn == "InstLdweights":
                    pending.append((len(fused_out), inst))
                    fused_out.append(inst)
                    continue
                if tn == "InstMatmult" and getattr(inst, "ldweights", None) is not True \
                        and pending:
                    want = str(inst.ins[1])
                    hit = next((k for k, (_, l) in enumerate(pending)
                                if str(l.ins[0]) == want), None)
                    if hit is not None:
                        pos, ldw = pending.pop(hit)
                        fused_out[pos] = None  # remove the standalone LDW
                        # pending positions after pos stay valid via None-skip
                        inst.ldweights = True
                        lsi = getattr(ldw, "sync_info", None)
                        if lsi is not None and (lsi.on_wait or lsi.on_update):
                            si = inst.sync_info
                            if si is None:
                                inst.sync_info = mybir.SyncInfo(
                                    on_wait=list(lsi.on_wait or []),
                                    on_update=list(lsi.on_update or []),
                                )
                            else:
                                si.on_wait = list(lsi.on_wait or []) + \
                                    list(si.on_wait or [])
                                si.on_update = list(si.on_update or []) + \
                                    list(lsi.on_update or [])
                    # no exact match: leave both the LDW and the MM as-is
                fused_out.append(inst)
            insts[:] = [i for i in fused_out if i is not None]

        # Sweep 1b: batch per-engine-clock semaphore increments. Every
        # instruction carries a unit inc of its engine clock sem; the EVT
        # register writes serialize (~26ns each) and back up behind a dense
        # burst, delaying dependent wait releases by hundreds of ns. Only
        # counter values some instruction actually WAITS on need to be
        # externally visible, so coalesce runs of unit incs onto the
        # instruction that reaches each waited threshold (sem-add-imm with
        # the accumulated value). Waits use >=, so overshooting emission
        # points is never needed; exact thresholds are preserved.
        if os.environ.get("BASS_BATCH_INCS", "0") == "1":
            waited = {}  # sem id -> sorted list of waited values
            unbatchable = set()  # sems with register-valued waits
            for insts in ordered.values():
                for inst in insts:
                    si = getattr(inst, "sync_info", None)
                    if si is None or not si.on_wait:
                        continue
                    for w in si.on_wait:
                        sid = getattr(w, "id", None)
                        val = getattr(w, "wait_value", None)
                        if sid is None:
                            continue
                        if val is None or getattr(w, "wait_reg", None):
                            unbatchable.add(sid)
                        else:
                            waited.setdefault(sid, set()).add(val)
            waited = {k: sorted(v) for k, v in waited.items()
                      if k not in unbatchable}
            import bisect
            for insts in ordered.values():
                counters = {}
                pending = {}
                last_upd = {}
                for inst in insts:
                    si = getattr(inst, "sync_info", None)
                    if si is None or not si.on_update:
                        continue
                    keep = []
                    for u in si.on_update:
                        if (str(u.update_mode).endswith("sem-inc")
                                and u.update_value in (None, 1)
                                and not getattr(u, "update_reg", None)
                                and u.id in waited):
                            v = counters.get(u.id, 0) + 1
                            counters[u.id] = v
                            pending[u.id] = pending.get(u.id, 0) + 1
                            ws = waited[u.id]
                            i = bisect.bisect_left(ws, v)
                            hit = i < len(ws) and ws[i] == v
                            if hit:
                                u.update_value = pending[u.id]
                                if pending[u.id] > 1:
                                    u.update_mode = "sem-add-imm"
                                pending[u.id] = 0
                                keep.append(u)
                                last_upd[u.id] = (inst, u)
                            # else: drop this inc, carried by a later one
                        else:
                            keep.append(u)
                    si.on_update = keep
                # flush any trailing pendings onto the last kept update (or
                # the final instruction) so end-of-stream totals match the
                # drain barrier's expectations.
                for sid, p in pending.items():
                    if p <= 0:
                        continue
                    if sid in last_upd:
                        # append p to the final update's value via a fresh
                        # update on the last instruction of the stream
                        pass
                    tgt = insts[-1]
                    tsi = getattr(tgt, "sync_info", None)
                    if tsi is None:
                        tgt.sync_info = tsi = mybir.SyncInfo(on_wait=[],
                                                             on_update=[])
                    appended = False
                    for u in tsi.on_update:
                        if getattr(u, "id", None) == sid:
                            u.update_value = (u.update_value or 1) + p
                            appended = True
                            break
                    if not appended and sid in last_upd:
                        _, u0 = last_upd[sid]
                        u0.update_value = (u0.update_value or 1) + p
                        if u0.update_value > 1:
                            u0.update_mode = "sem-add-imm"

        # Sweep 2: spill extra sync-waits onto preceding same-engine NOPs.
        for insts in ordered.values():
            out = []
            for inst in insts:
                si = getattr(inst, "sync_info", None)
                eng = getattr(inst, "engine", None)
                if si is not None and eng is not None and si.on_wait is not None \
                        and len(si.on_wait) > 1:
                    waits = list(si.on_wait)
                    for w in waits[:-1]:
                        out.append(mybir.InstNoOp(
                            name=nc.get_next_instruction_name(),
                            engine=eng,
                            bass_nofuse=True,
                            sync_info=mybir.SyncInfo(on_wait=[w], on_update=[]),
                        ))
                    si.on_wait = waits[-1:]
                out.append(inst)
            insts[:] = out
        return _orig_lower(self, ordered)

    tile.TileContext._lower_ordered_insts = _split_multi_waits


_patch_tile_drain()

F32 = mybir.dt.float32
BF16 = mybir.dt.bfloat16
FP8 = mybir.dt.float8e4
BF = ml_dtypes.bfloat16
F8 = (ml_dtypes.float8_e4m3fn if hasattr(ml_dtypes, "float8_e4m3fn")
      else ml_dtypes.float8_e4m3)
AF = mybir.ActivationFunctionType
ALU = mybir.AluOpType

HID = 512
INP = 512
K = 49
L = 49
VOCAB = 10000
B, T = 32, 64
NCORES = 8
BL = B // NCORES          # 4 sequences per core
NBT = BL * T              # 256 (t-major: col = t*BL + b)
G4 = 4 * HID              # 2048
NKC = HID // 128          # 4 k-chunks
NMT = G4 // 128           # 16 gate m-tiles
NCH = 20                  # vocab chunks
CHN = VOCAB // NCH        # 500
YGRP = 5                  # vocab chunks per y-store descriptor

# gate block layout (host perm = [g, i, f, o]); columns of act tiles
GB = slice(0, 16)         # tanh(g)
IB = slice(16, 32)        # sigmoid(i)
FB = slice(32, 48)        # sigmoid(f)
OB = slice(48, 64)        # sigmoid(o)

LAST_RESULT = None        # BassKernelResults of the most recent run (for test.py)


def _bcast_ap(ap2d, count, pos=1):
    """Insert a zero-stride dim of `count` at free position `pos` of a 2-D AP."""
    dims = list(ap2d.ap)
    dims.insert(pos, [0, count])
    return bass.AP(tensor=ap2d.tensor, offset=ap2d.offset, ap=dims)


def _part_bcast(dram_ap, parts):
    """DRAM AP replicated over `parts` partitions (zero-stride partition dim)."""
    return bass.AP(
        tensor=dram_ap.tensor, offset=dram_ap.offset,
        ap=[[0, parts]] + list(dram_ap.ap),
    )


def _kc_view(drm, rows, cols, nkc):
    """DRAM AP for [rows=nkc*128, cols] viewed as [128(part), nkc, cols]."""
    ap = drm[:]
    return bass.AP(tensor=ap.tensor, offset=ap.offset,
                   ap=[[cols, 128], [cols * 128, nkc], [1, cols]])


def _build():
    nc = bass.Bass()

    d_xT = nc.dram_tensor("xT", [INP, NBT], BF16, kind="ExternalInput")
    d_vT = nc.dram_tensor("vT", [HID, BL * L], BF16, kind="ExternalInput")
    d_vnat = nc.dram_tensor("vnat", [BL * L, HID], BF16, kind="ExternalInput")
    d_wihT = nc.dram_tensor("wihT", [INP, G4], BF16, kind="ExternalInput")
    d_whhT = nc.dram_tensor("whhT", [HID, G4], FP8, kind="ExternalInput")
    d_wxsT = nc.dram_tensor("wxsT", [INP, HID], BF16, kind="ExternalInput")
    d_whsT = nc.dram_tensor("whsT", [HID, HID], BF16, kind="ExternalInput")
    d_wgT = nc.dram_tensor("wgT", [HID, K], BF16, kind="ExternalInput")
    d_wsT = nc.dram_tensor("wsT", [HID, K], BF16, kind="ExternalInput")
    d_wvT = nc.dram_tensor("wvT", [HID, K], BF16, kind="ExternalInput")
    d_wmlpT = nc.dram_tensor("wmlpT", [HID, VOCAB], BF16, kind="ExternalInput")
    d_b4 = nc.dram_tensor("b4", [128, NMT], F32, kind="ExternalInput")
    d_wh = nc.dram_tensor("wh", [K], BF16, kind="ExternalInput")
    d_y = nc.dram_tensor("y", [NBT, VOCAB], BF16, kind="ExternalOutput")

    ident = nc.inline_tensor(np.eye(128, dtype=np.float32), name="ident128")
    ident_bf = nc.inline_tensor(np.eye(128, dtype=ml_dtypes.bfloat16),
                                name="ident128bf")

    from contextlib import ExitStack

    with tile.TileContext(nc) as tc, ExitStack() as es:
        consts = es.enter_context(tc.tile_pool(name="consts", bufs=1))
        state = es.enter_context(tc.tile_pool(name="state", bufs=1))
        dram = es.enter_context(tc.tile_pool(name="dram", bufs=1, space="DRAM"))

        # ---- loads: one descriptor per tensor, 2 queues, ACT kept free ---
        def sbuf_kc(name, cols, dt=BF16):
            return consts.tile([128, NKC, cols], dt, tag=name, name=name)

        id_bf = consts.tile([128, 128], BF16, tag="identbf")
        nc.gpsimd.dma_start(out=id_bf[:], in_=ident_bf[:])
        id_sb = consts.tile([128, 128], F32, tag="ident")
        nc.gpsimd.dma_start(out=id_sb[:], in_=ident[:])
        xT = sbuf_kc("xT", NBT)
        nc.sync.dma_start(out=xT[:], in_=_kc_view(d_xT, INP, NBT, NKC))
        wihT = sbuf_kc("wihT", G4)
        # split big weights across both queues (halves of the kc dim)
        nc.gpsimd.dma_start(
            out=wihT[:, 0:2, :],
            in_=bass.AP(tensor=d_wihT[:].tensor, offset=0,
                        ap=[[G4, 128], [G4 * 128, 2], [1, G4]]))
        nc.sync.dma_start(
            out=wihT[:, 2:4, :],
            in_=bass.AP(tensor=d_wihT[:].tensor, offset=2 * 128 * G4,
                        ap=[[G4, 128], [G4 * 128, 2], [1, G4]]))
        whhT = sbuf_kc("whhT", G4, FP8)
        nc.scalar.dma_start(
            out=whhT[:, 0:2, :],
            in_=bass.AP(tensor=d_whhT[:].tensor, offset=0,
                        ap=[[G4, 128], [G4 * 128, 2], [1, G4]]))
        nc.scalar.dma_start(
            out=whhT[:, 2:4, :],
            in_=bass.AP(tensor=d_whhT[:].tensor, offset=2 * 128 * G4,
                        ap=[[G4, 128], [G4 * 128, 2], [1, G4]]))
        b4 = consts.tile([128, NMT], F32, tag="b4")
        nc.sync.dma_start(out=b4[:], in_=d_b4[:])

        wxsT = sbuf_kc("wxsT", HID)
        nc.gpsimd.dma_start(out=wxsT[:], in_=_kc_view(d_wxsT, INP, HID, NKC))
        whsT = sbuf_kc("whsT", HID)
        nc.sync.dma_start(out=whsT[:], in_=_kc_view(d_whsT, HID, HID, NKC))
        wgT = sbuf_kc("wgT", K)
        nc.gpsimd.dma_start(out=wgT[:], in_=_kc_view(d_wgT, HID, K, NKC))
        wsT = sbuf_kc("wsT", K)
        nc.sync.dma_start(out=wsT[:], in_=_kc_view(d_wsT, HID, K, NKC))
        wvT = sbuf_kc("wvT", K)
        nc.gpsimd.dma_start(out=wvT[:], in_=_kc_view(d_wvT, HID, K, NKC))
        vT = sbuf_kc("vT", BL * L)
        nc.sync.dma_start(out=vT[:], in_=_kc_view(d_vT, HID, BL * L, NKC))

        vnat = consts.tile([L, BL, HID], BF16, tag="vnat")
        nc.gpsimd.dma_start(
            out=vnat[:],
            in_=bass.AP(tensor=d_vnat[:].tensor, offset=0,
                        ap=[[HID, L], [HID * L, BL], [1, HID]]))

        wh_rep = consts.tile([128, K], BF16, tag="wh_rep")
        nc.sync.dma_start(out=wh_rep[:], in_=_part_bcast(d_wh[:], 128))
        half1 = consts.tile([1, 128], F32, tag="half1")
        nc.vector.memset(half1[:], 0.5)   # beta broadcast carries the 0.5 of
                                          # the sentinel (tanh+1)/2 folding

        # Wmlp resident in SBUF; the 4 big DMAs are issued from inside the
        # phase-B loop (steps 26..38) so the HBM/SBUF-write traffic misses
        # both the phase-A window and the late-B MLP-hoist window.
        wmlp = consts.tile([128, NKC, VOCAB], BF16, tag="wmlp")

        def _wmlp_load(kc):
            nc.sync.dma_start(out=wmlp[:, kc, :],
                              in_=d_wmlpT[kc * 128 : (kc + 1) * 128, :])

        # ---- persistent state -------------------------------------------
        xg = state.tile([128, T, NMT, BL], BF16, tag="xg")
        xs = state.tile([128, NKC, T, BL], F32, tag="xs")
        tanhc = state.tile([128, NKC, T, BL], F32, tag="tanhc")
        h_bf = state.tile([128, NKC, T + 1, BL], BF16, tag="h_bf")
        c_st = state.tile([128, NKC * BL], F32, tag="c_st")
        vk_rep = state.tile([128, L, K], BF16, tag="vk_rep")
        s_f32 = state.tile([128, NKC, NBT], F32, tag="s_f32")
        s_bf = state.tile([128, NKC, NBT], BF16, tag="s_bf")
        u_bf = state.tile([128, NKC, NBT], BF16, tag="u_bf")
        hg_bf = state.tile([128, 2, K], BF16, tag="hg_bf")
        hg_f = state.tile([128, 2, K], F32, tag="hg_f")
        wh_f = state.tile([128, K], F32, tag="wh_f")
        zt_st = state.tile([128, 2, K], F32, tag="zt_st")
        ez_sb = state.tile([128, 2, K], F32, tag="ez_sb")
        ezs_f = state.tile([128, 2, K], F32, tag="ezs_f")
        ezT = state.tile([L, NBT], BF16, tag="ezT")
        scal = state.tile([128, 2, 8], F32, tag="scal")
        frow = state.tile([1, NBT], F32, tag="frow")
        fbc = state.tile([128, NBT], F32, tag="fbc")
        hh_f = state.tile([128, NKC, 128], F32, tag="hh_f")  # h cast, per-ti
        zin = state.tile([128, L, K], BF16, tag="zin")

        d_vk = dram.tile([BL * L, K], BF16, tag="d_vk")
        d_vkrep = dram.tile([128, L * K], BF16, tag="d_vkrep")

        nc.vector.memset(c_st[:], 0.0)
        nc.vector.memset(h_bf[:, :, 0, :], 0.0)
        nc.vector.tensor_copy(wh_f[:], wh_rep[:])

        # PSUM budget (8 banks): pg blocks 4 + aux 2 + mlp 2
        with tc.tile_pool(name="aux_ps", bufs=2, space="PSUM") as aux_ps, \
             tc.tile_pool(name="pb_psum", bufs=1, space="PSUM") as pb_psum, \
             tc.tile_pool(name="pb_tmp", bufs=3) as pb_tmp, \
             tc.tile_pool(name="mlp_ps", bufs=2, space="PSUM") as mlp_ps, \
             tc.tile_pool(name="pc_tmp", bufs=2) as pc_tmp, \
             tc.tile_pool(name="mlp_out", bufs=3) as mlp_out:

            def _bcast_col(ap_col, n1, n2):
                """[128,1] AP -> [128, n1, n2] zero-stride broadcast."""
                return bass.AP(tensor=ap_col.tensor, offset=ap_col.offset,
                               ap=[list(ap_col.ap[0]), [0, n1], [0, n2]])

            TH = T

            # ============ phase A: x_gates projection =====================
            def _xg_item(mt, th):
                # pg banks are idle until phase B; rotating phase A's GEMM
                # tiles across them (6 buffers total) removes the WAR stalls
                # of a 2-deep pipeline.
                r = mt % 6
                if r < 2:
                    p = aux_ps.tile([128, TH * BL], F32, tag="aux",
                                    name="pa")
                else:
                    p = pb_psum.tile([128, TH * BL], F32, tag=f"pg{r - 2}",
                                     name="pa")
                for kc in range(NKC):
                    nc.tensor.matmul(
                        p[:],
                        lhsT=wihT[:, kc, mt * 128 : (mt + 1) * 128],
                        rhs=xT[:, kc, th * TH * BL : (th + 1) * TH * BL],
                        start=(kc == 0),
                        stop=(kc == NKC - 1),
                    )
                # bias-add + copy to xg on DVE (ACT kept clear for the
                # phase-B chain; first-half copies run pre-B anyway)
                if th == 0 and mt % 2 == 0:
                    nc.scalar.activation(
                        out=xg[:, th * TH : (th + 1) * TH, mt, :],
                        in_=p[:].rearrange("p (t b) -> p t b", b=BL),
                        func=AF.Identity,
                        bias=b4[:, mt : mt + 1],
                        scale=1.0,
                    )
                else:
                    nc.vector.tensor_add(
                        xg[:, th * TH : (th + 1) * TH, mt, :],
                        p[:].rearrange("p (t b) -> p t b", b=BL),
                        _bcast_col(b4[:, mt : mt + 1], TH, BL),
                    )

            # PE warm-up (HAM un-throttle) + ACT sigmoid-table preload while
            # the weight DMAs stream in; id_sb arrives first on SWDGE.
            warm = state.tile([128, 2], F32, tag="warm")
            wps = aux_ps.tile([128, 128], F32, tag="aux")
            for w in range(48):
                nc.tensor.matmul(wps[:], lhsT=id_bf[:], rhs=id_bf[:],
                                 start=(w == 0), stop=(w == 47))
            nc.scalar.activation(out=warm[:, 0:1], in_=wps[:, 0:1],
                                 func=AF.Sigmoid)
            nc.scalar.activation(out=warm[:, 1:2], in_=wps[:, 0:1],
                                 func=AF.Tanh)

            for mt in range(NMT):
                _xg_item(mt, 0)

            # deferred work, one item per early phase-B step
            def _xs_item(st):
                p = aux_ps.tile([128, NBT], F32, tag="aux")
                for kc in range(NKC):
                    nc.tensor.matmul(
                        p[:],
                        lhsT=wxsT[:, kc, st * 128 : (st + 1) * 128],
                        rhs=xT[:, kc, :],
                        start=(kc == 0),
                        stop=(kc == NKC - 1),
                    )
                nc.vector.tensor_copy(
                    xs[:, st, :, :], p[:].rearrange("p (t b) -> p t b", b=BL))

            def _vk_item(half):
                rows = 128 if half == 0 else BL * L - 128
                p = aux_ps.tile([128, K], F32, tag="aux")
                for kc in range(NKC):
                    nc.tensor.matmul(
                        p[:rows, :],
                        lhsT=vT[:, kc, half * 128 : half * 128 + rows],
                        rhs=wvT[:, kc, :],
                        start=(kc == 0),
                        stop=(kc == NKC - 1),
                    )
                tmp = state.tile([128, K], BF16, tag=f"vkh{half}")
                nc.scalar.copy(out=tmp[:rows, :], in_=p[:rows, :])
                nc.gpsimd.dma_start(
                    out=d_vk[half * 128 : half * 128 + rows, :], in_=tmp[:rows, :])

            def _vk_bounce():
                src = d_vk[:]
                dst = d_vkrep[:]
                nc.gpsimd.dma_start(
                    out=bass.AP(tensor=dst.tensor, offset=dst.offset,
                                ap=[[L * K * BL, 128 // BL], [L * K, BL],
                                    [1, L * K]]),
                    in_=bass.AP(tensor=src.tensor, offset=src.offset,
                                ap=[[0, 128 // BL], [L * K, BL], [1, L * K]]),
                )

            def _vk_load():
                nc.gpsimd.dma_start(
                    out=vk_rep[:].rearrange("p l k -> p (l k)"), in_=d_vkrep[:])

            deferred = [lambda st=st: _xs_item(st) for st in range(NKC)]
            deferred += [lambda: _vk_item(0), lambda: _vk_item(1),
                         _vk_bounce, _vk_load]

            # ============== phase C: attention + sentinel =================
            # all tanh/exp (one ACT set); sentinel sigmoid folded into tanh.
            def c_pre_items(ti):
                cols = slice(1 + ti * 32, 1 + ti * 32 + 32)   # current h
                pcols = slice(0 + ti * 32, 0 + ti * 32 + 32)  # h_prev
                bt = slice(ti * 128, (ti + 1) * 128)

                items = []

                def _hg():
                    ph = aux_ps.tile([128, K], F32, tag="aux")
                    for kc in range(NKC):
                        nc.tensor.matmul(
                            ph[:], lhsT=h_bf[:, kc, cols, :], rhs=wgT[:, kc, :],
                            start=(kc == 0), stop=(kc == NKC - 1),
                        )
                    nc.vector.tensor_copy(hg_bf[:, ti, :], ph[:])
                    nc.scalar.copy(out=hg_f[:, ti, :], in_=ph[:])
                items.append(_hg)

                # z-chain in L slices (bf16); muls alternate GP/DVE
                def _z(idx, l0, l1):
                    nc.vector.tensor_add(
                        zin[:, l0:l1, :], vk_rep[:, l0:l1, :],
                        _bcast_ap(hg_bf[:, ti, :], l1 - l0),
                    )
                    nc.scalar.activation(
                        out=zin[:, l0:l1, :], in_=zin[:, l0:l1, :], func=AF.Tanh
                    )
                    nc.vector.tensor_mul(
                        zin[:, l0:l1, :], zin[:, l0:l1, :],
                        _bcast_ap(wh_rep[:], l1 - l0),
                    )
                    nc.vector.tensor_reduce(
                        out=zt_st[:, ti, l0:l1], in_=zin[:, l0:l1, :],
                        axis=mybir.AxisListType.X, op=ALU.add,
                    )
                zsl = ((0, 7), (7, 13), (13, 19), (19, 25),
                       (25, 31), (31, 37), (37, 43), (43, L)) if ti == 0 \
                    else ((0, 25), (25, L))
                for idx, (l0, l1) in enumerate(zsl):
                    items.append(lambda idx=idx, l0=l0, l1=l1: _z(idx, l0, l1))

                # sentinel: 2s = (tanh(x/2)+1) * tanh(c); Ws carries the 0.5
                def _sent(st):
                    ps = aux_ps.tile([128, 128], F32, tag="aux")
                    for kc in range(NKC):
                        nc.tensor.matmul(
                            ps[:], lhsT=whsT[:, kc, st * 128 : (st + 1) * 128],
                            rhs=h_bf[:, kc, pcols, :],
                            start=(kc == 0), stop=(kc == NKC - 1),
                        )
                    pssb = pc_tmp.tile([128, 128], F32, tag="pssb")
                    nc.vector.tensor_add(
                        pssb[:].rearrange("p (t b) -> p t b", b=BL),
                        ps[:].rearrange("p (t b) -> p t b", b=BL),
                        xs[:, st, ti * 32 : (ti + 1) * 32, :],
                    )
                    nc.scalar.activation(out=pssb[:], in_=pssb[:],
                                         func=AF.Tanh, scale=0.5)
                    nc.vector.scalar_tensor_tensor(
                        out=s_f32[:, st, bt].rearrange("p (t b) -> p t b", b=BL),
                        in0=pssb[:].rearrange("p (t b) -> p t b", b=BL),
                        scalar=1.0,
                        in1=tanhc[:, st, ti * 32 : (ti + 1) * 32, :],
                        op0=ALU.add, op1=ALU.mult,
                    )
                    nc.vector.tensor_copy(s_bf[:, st, bt], s_f32[:, st, bt])
                for st in range(NKC):
                    items.append(lambda st=st: _sent(st))

                # sentinel gate pre-activation: tanh((0.5 Ws)@ 2s + hg) . wh
                def _sws():
                    pw = aux_ps.tile([128, K], F32, tag="aux")
                    for kc in range(NKC):
                        nc.tensor.matmul(
                            pw[:], lhsT=s_bf[:, kc, bt], rhs=wsT[:, kc, :],
                            start=(kc == 0), stop=(kc == NKC - 1),
                        )
                    sws = pc_tmp.tile([128, K], F32, tag="sws")
                    nc.vector.tensor_add(sws[:], pw[:], hg_f[:, ti, :])
                    nc.scalar.activation(out=sws[:], in_=sws[:], func=AF.Tanh)
                    nc.vector.tensor_mul(sws[:], sws[:], wh_f[:])
                    nc.vector.tensor_reduce(
                        out=scal[:, ti, 1:2], in_=sws[:],
                        axis=mybir.AxisListType.X, op=ALU.add,
                    )
                items.append(_sws)
                return items

            def c_exp_part(ti):
                bt = slice(ti * 128, (ti + 1) * 128)
                nc.scalar.activation(
                    out=ez_sb[:, ti, :], in_=zt_st[:, ti, :], func=AF.Exp,
                    accum_out=scal[:, ti, 0:1],
                )
                nc.scalar.activation(out=scal[:, ti, 1:2], in_=scal[:, ti, 1:2],
                                     func=AF.Exp)
                nc.vector.tensor_add(scal[:, ti, 2:3], scal[:, ti, 0:1],
                                     scal[:, ti, 1:2])
                nc.vector.reciprocal(scal[:, ti, 3:4], scal[:, ti, 2:3])
                nc.vector.tensor_mul(scal[:, ti, 4:5], scal[:, ti, 1:2],
                                     scal[:, ti, 3:4])
                nc.vector.tensor_scalar_mul(ezs_f[:, ti, :], ez_sb[:, ti, :],
                                            scal[:, ti, 3:4])
                pt = aux_ps.tile([K, 128], F32, tag="aux")
                nc.tensor.transpose(pt[:], ezs_f[:, ti, :], id_sb[:])
                nc.vector.tensor_copy(ezT[:, bt], pt[:])
                pf = aux_ps.tile([1, 128], F32, tag="aux")
                nc.tensor.transpose(pf[:], scal[:, ti, 4:5], id_sb[:])
                nc.vector.tensor_copy(frow[:, bt], pf[:])
                pfb = aux_ps.tile([128, 128], F32, tag="aux")
                # half1 (0.5) carries the sentinel folding into beta
                nc.tensor.matmul(pfb[:], lhsT=half1[:], rhs=frow[:, bt],
                                 start=True, stop=True)
                nc.vector.tensor_copy(fbc[:, bt], pfb[:])

                for hc in range(NKC):
                    nc.vector.tensor_copy(
                        hh_f[:, hc, :],
                        h_bf[:, hc, 1 + ti * 32 : 1 + ti * 32 + 32, :]
                        .rearrange("p t b -> p (t b)"),
                    )
                    pc = aux_ps.tile([128, 128], F32, tag="aux")
                    for b in range(BL):
                        nc.tensor.matmul(
                            pc[:].rearrange("p (t b) -> p t b", b=BL)[:, :, b],
                            lhsT=vnat[:, b, hc * 128 : (hc + 1) * 128],
                            rhs=ezT[:, bt].rearrange("p (t b) -> p t b", b=BL)[:, :, b],
                            start=True, stop=True,
                        )
                    us = pc_tmp.tile([128, 128], F32, tag="us")
                    nc.vector.tensor_mul(us[:], s_f32[:, hc, bt], fbc[:, bt])
                    nc.vector.tensor_add(us[:], us[:], pc[:])
                    nc.vector.tensor_add(us[:], us[:], hh_f[:, hc, :])
                    nc.vector.tensor_copy(u_bf[:, hc, bt], us[:])

            ygrp_state = {}

            def mlp_chunk(ti, nch):
                g0 = (nch // YGRP) * YGRP
                key = (ti, g0)
                if key not in ygrp_state:
                    ygrp_state[key] = mlp_out.tile([128, YGRP * CHN], BF16,
                                                   tag="ysb", name="ysb")
                ysb = ygrp_state[key]
                j = nch - g0
                if ti == 1 and nch % 6 >= 2:
                    # post-recurrence the pg banks are idle: rotating the
                    # accumulators across them (6-deep) keeps the MLP purely
                    # PE-bound instead of waiting on 2-банк cast turnaround
                    py = pb_psum.tile([128, CHN], F32,
                                      tag=f"pg{nch % 6 - 2}", name="py")
                else:
                    py = mlp_ps.tile([128, CHN], F32, tag="py")
                for kc in range(NKC):
                    nc.tensor.matmul(
                        py[:],
                        lhsT=u_bf[:, kc, ti * 128 : (ti + 1) * 128],
                        rhs=wmlp[:, kc, nch * CHN : (nch + 1) * CHN],
                        start=(kc == 0), stop=(kc == NKC - 1),
                    )
                if ti == 0:
                    # hoisted into phase B: half-width DVE casts block the
                    # recurrence chain for at most ~0.26us each
                    h = CHN // 2
                    nc.vector.tensor_copy(
                        ysb[:, j * CHN : j * CHN + h], py[:, 0:h])
                    nc.vector.tensor_copy(
                        ysb[:, j * CHN + h : (j + 1) * CHN], py[:, h:CHN])
                elif nch % 2 == 0:
                    nc.vector.tensor_copy(ysb[:, j * CHN : (j + 1) * CHN],
                                          py[:])
                else:
                    nc.scalar.copy(out=ysb[:, j * CHN : (j + 1) * CHN],
                                   in_=py[:])
                if j == YGRP - 1:
                    # all y stores on the SP queue: the SWDGE (gpsimd) ring
                    # then empties long before the tail, so the final
                    # all-engine barrier doesn't wait on its ~4us drain.
                    nc.sync.dma_start(
                        out=d_y[ti * 128 : (ti + 1) * 128,
                                g0 * CHN : (g0 + YGRP) * CHN],
                        in_=ysb[:],
                    )

            def mlp_ti(ti):
                for nch in range(NCH):
                    mlp_chunk(ti, nch)


            # ============== phase B: serial LSTM recurrence ===============
            # burst tile order [g, i, f, o]; chain overlaps the burst tail.
            for t in range(T):
                pgs = [pb_psum.tile([128, NKC * BL], F32, tag=f"pg{blk}",
                                    name=f"pg{blk}")
                       for blk in range(4)]
                prev_mm = None
                for blk in range(4):
                    for sub in range(4):
                        mt = blk * 4 + sub
                        for kc in range(NKC):
                            # start=True clears has_written for the WHOLE
                            # bank, so only the block's first matmul may set
                            # it; later sub-tiles first-write via the cleared
                            # has_written bits (overwrite) then accumulate.
                            mm = nc.tensor.matmul(
                                pgs[blk][:, sub * BL : (sub + 1) * BL],
                                lhsT=whhT[:, kc, mt * 128 : (mt + 1) * 128],
                                rhs=h_bf[:, kc, t, :],
                                start=(sub == 0 and kc == 0),
                                stop=False,
                                skip_group_check=True,
                            )
                            if prev_mm is not None:
                                add_dep_helper(mm.ins, prev_mm.ins, sync=False,
                                               reason="psum group order")
                            prev_mm = mm
                    # fold the +xg term into the accumulation with a bf16
                    # identity matmul; the gate activation then reads PSUM
                    # directly (no DVE add on the critical path).
                    mm = nc.tensor.matmul(
                        pgs[blk][:],
                        lhsT=id_bf[:],
                        rhs=xg[:, t, blk * 4 : blk * 4 + 4, :]
                        .rearrange("p m b -> p (m b)"),
                        start=False, stop=True, skip_group_check=True,
                    )
                    add_dep_helper(mm.ins, prev_mm.ins, sync=False,
                                   reason="psum group order")
                    prev_mm = mm

                act = pb_tmp.tile([128, NMT * BL], F32, tag="act")
                act3 = act[:].rearrange("p (m b) -> p m b", b=BL)

                # ACT order: tanh_g, sig_i, sig_f, sig_o, tanh_c (PSUM src)
                # DVE order: cmul, cadd, hmul; ig on GpSimd
                nc.scalar.activation(out=act[:, GB], in_=pgs[0][:],
                                     func=AF.Tanh)
                nc.scalar.activation(out=act[:, IB], in_=pgs[1][:],
                                     func=AF.Sigmoid)
                nc.scalar.activation(out=act[:, FB], in_=pgs[2][:],
                                     func=AF.Sigmoid)

                ig = pb_tmp.tile([128, NKC * BL], F32, tag="ig")
                nc.gpsimd.tensor_mul(ig[:], act[:, IB], act[:, GB])
                nc.vector.tensor_mul(c_st[:], act[:, FB], c_st[:])
                nc.scalar.activation(out=act[:, OB], in_=pgs[3][:],
                                     func=AF.Sigmoid)
                nc.vector.tensor_add(c_st[:], c_st[:], ig[:])
                nc.scalar.activation(
                    out=tanhc[:, :, t, :],
                    in_=c_st[:].rearrange("p (k b) -> p k b", b=BL),
                    func=AF.Tanh,
                )
                nc.vector.tensor_mul(
                    h_bf[:, :, t + 1, :],
                    act3[:, 12:16, :],
                    tanhc[:, :, t, :],
                )

                if t < len(deferred):
                    deferred[t]()
                if t >= 26 and t % 4 == 2 and (t - 26) // 4 < NKC:
                    _wmlp_load((t - 26) // 4)

            for it in c_pre_items(0):
                it()
            c_exp_part(0)
            mlp_ti(0)
            for it in c_pre_items(1):
                it()
            c_exp_part(1)
            mlp_ti(1)

    return nc


_NC_CACHE = None


def kernel(**inputs):
    global _NC_CACHE, LAST_RESULT
    x = np.asarray(inputs["x"], np.float32)
    V = np.asarray(inputs["V"], np.float32)
    Wih = np.asarray(inputs["Wih"], np.float32)
    Whh = np.asarray(inputs["Whh"], np.float32)
    bih = np.asarray(inputs["bih"], np.float32)
    bhh = np.asarray(inputs["bhh"], np.float32)
    Wx_s = np.asarray(inputs["Wx_s"], np.float32)
    Wh_s = np.asarray(inputs["Wh_s"], np.float32)
    Wv = np.asarray(inputs["Wv"], np.float32)
    Wg = np.asarray(inputs["Wg"], np.float32)
    Wh_att = np.asarray(inputs["Wh_att"], np.float32)
    Ws = np.asarray(inputs["Ws"], np.float32)
    Wmlp = np.asarray(inputs["Wmlp"], np.float32)
    bmlp = np.asarray(inputs["bmlp"], np.float32)

    if _NC_CACHE is None:
        _NC_CACHE = _build()
    nc = _NC_CACHE

    # permute gate order [i,f,g,o] -> [g,i,f,o] (see phase B chain schedule)
    perm = np.concatenate([
        np.arange(1024, 1536), np.arange(0, 512),
        np.arange(512, 1024), np.arange(1536, 2048)
    ])
    shared = {
        "wihT": np.ascontiguousarray(Wih.T[:, perm]).astype(BF),
        "whhT": np.ascontiguousarray(Whh.T[:, perm]).astype(F8),
        "wxsT": np.ascontiguousarray(Wx_s.T).astype(BF),
        "whsT": np.ascontiguousarray(Wh_s.T).astype(BF),
        "wgT": np.ascontiguousarray(Wg.T).astype(BF),
        # sentinel sigmoid -> (tanh(x/2)+1)/2: s is computed scaled by 2,
        # compensated by 0.5 here and in the beta broadcast (half1)
        "wsT": np.ascontiguousarray(0.5 * Ws.T).astype(BF),
        "wvT": np.ascontiguousarray(Wv.T).astype(BF),
        "wmlpT": np.ascontiguousarray(Wmlp.T).astype(BF),
        "b4": np.ascontiguousarray((bih + bhh)[perm].reshape(NMT, 128).T),
        "wh": np.ascontiguousarray(Wh_att[0]).astype(BF),
    }
    in_maps = []
    for c in range(NCORES):
        xi = x[c * BL : (c + 1) * BL]          # [BL, T, INP]
        Vi = V[c * BL : (c + 1) * BL]          # [BL, L, HID]
        xT = np.ascontiguousarray(xi.transpose(2, 1, 0).reshape(INP, NBT)).astype(BF)
        vflat = Vi.reshape(BL * L, HID)
        in_maps.append(dict(shared,
                            xT=xT,
                            vT=np.ascontiguousarray(vflat.T).astype(BF),
                            vnat=np.ascontiguousarray(vflat).astype(BF)))

    trace = os.environ.get("BASS_KERNEL_TRACE", "0") == "1"
    res = run_bass_kernel_spmd(nc, in_maps, core_ids=list(range(NCORES)), trace=trace)
    LAST_RESULT = res

    out = np.empty((B, T, VOCAB), np.float32)
    for c in range(NCORES):
        yc = res.results[c]["y"].astype(np.float32).reshape(T, BL, VOCAB)
        out[c * BL : (c + 1) * BL] = yc.transpose(1, 0, 2)
    out += bmlp
    return out


# revision 52
# speedup vs baseline: 1.0094x; 1.0094x over previous
"""Adaptive-attention LSTM (B=32,T=64,HID=512,K=L=49,VOCAB=10000) on 8 TRN2 cores.

Strategy (v3)
-------------
Data-parallel over batch: each core gets B_local=4 sequences, feature-on-
partition / (t,b)-on-free layout throughout.

  loads:   one coalesced strided DMA descriptor per tensor (descriptor issue
           costs ~0.7us of engine time), on the SP+SWDGE queues only so the
           ACT engine stays free; Wmlp preloaded whole into SBUF (80KB/
           partition) so the MLP never touches HBM.
  phase A: x_gates = Wih@x + bias for t<32 up front; the t>=32 half plus the
           xs/Vk projections issue one-per-step inside phase B's PE gaps.
  phase B: 64-step serial LSTM. Gate blocks ordered [g,i,f,o] with one PSUM
           tile per block so each block's +xg add / activation starts as soon
           as that block's matmuls retire (the burst is LDW-bound; the chain
           hides under its tail). h written directly as bf16.
  phase C: attention + sentinel, all in tanh/exp form (the sentinel sigmoid
           is computed as (tanh(x/2)+1)/2 with the 0.5's folded into Ws and
           the beta broadcast), so phases B and C each use a single ACT
           table set and the scheduler may interleave phase C and the MLP
           into idle engine slots without table thrash. Exact folded softmax
           u = (e_how*s + e_z@V)/D + h with D = sum(e_z)+e_how.
  MLP:     y = u @ Wmlp.T, PSUM->bf16 casts alternating DVE/ACT, stores
           batched 5 vocab-chunks per descriptor; bmlp added on host.
"""

import os
import sys
import types

for _p in ("/opt/pypackages", "/opt/trn_rl_repo"):
    if _p not in sys.path and os.path.isdir(_p):
        sys.path.insert(0, _p)


def _install_ntff_shim():
    """antenv.axon_hooks is missing in the trimmed repo; provide it so
    run_bass_kernel_spmd(trace=True) can reach the NTFF profile hook."""
    if "antenv.axon_hooks" in sys.modules:
        return
    try:
        from trn_agent_boot.trn_boot import _ntff_profile_via_ctypes

        hook = _ntff_profile_via_ctypes("/opt/axon/libaxon_pjrt.so")
    except Exception:
        hook = None
    m = types.ModuleType("antenv.axon_hooks")
    m.get_axon_ntff_profile_hook = lambda: hook
    m.set_axon_ntff_profile_hook = lambda h: None
    sys.modules["antenv.axon_hooks"] = m


_install_ntff_shim()


def _enable_ldw_opt():
    """compile flags hardcode --enable-ldw-opt=false; flip it so LDWEIGHTS can
    use the fast-weight-load path."""
    import concourse.bass_utils as bu

    if os.environ.get("BASS_LDW_OPT", "1") != "1":
        return
    if getattr(bu, "_ldw_patched", False):
        return
    orig = bu.run_command

    def patched(argv, **kw):
        argv = [a.replace("--enable-ldw-opt=false", "--enable-ldw-opt=true")
                if isinstance(a, str) else a for a in argv]
        return orig(argv, **kw)

    bu.run_command = patched
    bu._ldw_patched = True


_enable_ldw_opt()

import ml_dtypes
import numpy as np

import concourse.bass as bass
import concourse.mybir as mybir
import concourse.tile as tile
from concourse.bass_utils import run_bass_kernel_spmd
from concourse.tile import add_dep_helper
from concourse.vector_clock import ScopedClock


def _patch_tile_drain():
    """This walrus build allows a single sync-wait per CTRL instruction; Tile's
    tail drain attaches one wait per live semaphore. Spread them over a chain
    of SP nops (sequential on SP -> identical semantics)."""
    if getattr(tile.TileContext, "_drain_patched", False):
        return

    def _drain_and_barrier(self, tick_clock, wait_clock):
        nc = self.nc
        probe = nc.sync.nop(nofuse=True)
        wait_clock.add_sem_waits(
            probe.ins, ScopedClock({None: tick_clock.global_clock})
        )
        waits = list(probe.ins.sync_info.on_wait)
        if len(waits) > 1:
            probe.ins.sync_info.on_wait = waits[:1]
            for i in range(1, len(waits)):
                extra = nc.sync.nop(nofuse=True)
                if extra.ins.sync_info is None:
                    extra.ins.sync_info = mybir.SyncInfo(
                        on_wait=waits[i : i + 1], on_update=[]
                    )
                else:
                    extra.ins.sync_info.on_wait = waits[i : i + 1]
        nc.sync.drain()
        nc.all_engine_barrier()
        assert self.sems is not None
        popped = nc._tile_sem_poison_stack.pop()
        assert popped is self._sem_poison
        nc.clear_and_free_semaphores(list(self.sems.allocated().values()))
        nc.all_engine_barrier()

    tile.TileContext._drain_and_barrier = _drain_and_barrier
    tile.TileContext._drain_patched = True

    # General pass: the ISA here allows a single sync-wait per instruction.
    # Before lowering, split any instruction with N>1 waits into N-1 preceding
    # single-wait NOPs on the same engine (engine streams execute in order, so
    # semantics are identical).
    _orig_lower = tile.TileContext._lower_ordered_insts

    def _split_multi_waits(self, ordered):
        nc = self.nc
        # Sweep 1: fuse standalone InstLdweights back into their InstMatmult
        # (self-loading form, ldweights=True). The standalone form defeats the
        # codegen fast-weight-load path. Pair FIFO by PE stream order; merge
        # sync lists.
        fuse = os.environ.get("BASS_FUSE_LDW", "1") == "1"
        for insts in ordered.values():
            if not fuse:
                break
            fused_out = []
            pending = []  # (index in fused_out, ldw inst)
            for inst in insts:
                tn = type(inst).__name__
                if tn == "InstLdweights":
                    pending.append((len(fused_out), inst))
                    fused_out.append(inst)
                    continue
                if tn == "InstMatmult" and getattr(inst, "ldweights", None) is not True \
                        and pending:
                    want = str(inst.ins[1])
                    hit = next((k for k, (_, l) in enumerate(pending)
                                if str(l.ins[0]) == want), None)
                    if hit is not None:
                        pos, ldw = pending.pop(hit)
                        fused_out[pos] = None  # remove the standalone LDW
                        # pending positions after pos stay valid via None-skip
                        inst.ldweights = True
                        lsi = getattr(ldw, "sync_info", None)
                        if lsi is not None and (lsi.on_wait or lsi.on_update):
                            si = inst.sync_info
                            if si is None:
                                inst.sync_info = mybir.SyncInfo(
                                    on_wait=list(lsi.on_wait or []),
                                    on_update=list(lsi.on_update or []),
                                )
                            else:
                                si.on_wait = list(lsi.on_wait or []) + \
                                    list(si.on_wait or [])
                                si.on_update = list(si.on_update or []) + \
                                    list(lsi.on_update or [])
                    # no exact match: leave both the LDW and the MM as-is
                fused_out.append(inst)
            insts[:] = [i for i in fused_out if i is not None]

        # Sweep 1b: batch per-engine-clock semaphore increments. Every
        # instruction carries a unit inc of its engine clock sem; the EVT
        # register writes serialize (~26ns each) and back up behind a dense
        # burst, delaying dependent wait releases by hundreds of ns. Only
        # counter values some instruction actually WAITS on need to be
        # externally visible, so coalesce runs of unit incs onto the
        # instruction that reaches each waited threshold (sem-add-imm with
        # the accumulated value). Waits use >=, so overshooting emission
        # points is never needed; exact thresholds are preserved.
        if os.environ.get("BASS_BATCH_INCS", "0") == "1":
            waited = {}  # sem id -> sorted list of waited values
            unbatchable = set()  # sems with register-valued waits
            for insts in ordered.values():
                for inst in insts:
                    si = getattr(inst, "sync_info", None)
                    if si is None or not si.on_wait:
                        continue
                    for w in si.on_wait:
                        sid = getattr(w, "id", None)
                        val = getattr(w, "wait_value", None)
                        if sid is None:
                            continue
                        if val is None or getattr(w, "wait_reg", None):
                            unbatchable.add(sid)
                        else:
                            waited.setdefault(sid, set()).add(val)
            waited = {k: sorted(v) for k, v in waited.items()
                      if k not in unbatchable}
            import bisect
            for insts in ordered.values():
                counters = {}
                pending = {}
                last_upd = {}
                for inst in insts:
                    si = getattr(inst, "sync_info", None)
                    if si is None or not si.on_update:
                        continue
                    keep = []
                    for u in si.on_update:
                        if (str(u.update_mode).endswith("sem-inc")
                                and u.update_value in (None, 1)
                                and not getattr(u, "update_reg", None)
                                and u.id in waited):
                            v = counters.get(u.id, 0) + 1
                            counters[u.id] = v
                            pending[u.id] = pending.get(u.id, 0) + 1
                            ws = waited[u.id]
                            i = bisect.bisect_left(ws, v)
                            hit = i < len(ws) and ws[i] == v
                            if hit:
                                u.update_value = pending[u.id]
                                if pending[u.id] > 1:
                                    u.update_mode = "sem-add-imm"
                                pending[u.id] = 0
                                keep.append(u)
                                last_upd[u.id] = (inst, u)
                            # else: drop this inc, carried by a later one
                        else:
                            keep.append(u)
                    si.on_update = keep
                # flush any trailing pendings onto the last kept update (or
                # the final instruction) so end-of-stream totals match the
                # drain barrier's expectations.
                for sid, p in pending.items():
                    if p <= 0:
                        continue
                    if sid in last_upd:
                        # append p to the final update's value via a fresh
                        # update on the last instruction of the stream
                        pass
                    tgt = insts[-1]
                    tsi = getattr(tgt, "sync_info", None)
                    if tsi is None:
                        tgt.sync_info = tsi = mybir.SyncInfo(on_wait=[],
                                                             on_update=[])
                    appended = False
                    for u in tsi.on_update:
                        if getattr(u, "id", None) == sid:
                            u.update_value = (u.update_value or 1) + p
                            appended = True
                            break
                    if not appended and sid in last_upd:
                        _, u0 = last_upd[sid]
                        u0.update_value = (u0.update_value or 1) + p
                        if u0.update_value > 1:
                            u0.update_mode = "sem-add-imm"

        # Sweep 2: spill extra sync-waits onto preceding same-engine NOPs.
        for insts in ordered.values():
            out = []
            for inst in insts:
                si = getattr(inst, "sync_info", None)
                eng = getattr(inst, "engine", None)
                if si is not None and eng is not None and si.on_wait is not None \
                        and len(si.on_wait) > 1:
                    waits = list(si.on_wait)
                    for w in waits[:-1]:
                        out.append(mybir.InstNoOp(
                            name=nc.get_next_instruction_name(),
                            engine=eng,
                            bass_nofuse=True,
                            sync_info=mybir.SyncInfo(on_wait=[w], on_update=[]),
                        ))
                    si.on_wait = waits[-1:]
                out.append(inst)
            insts[:] = out
        return _orig_lower(self, ordered)

    tile.TileContext._lower_ordered_insts = _split_multi_waits


_patch_tile_drain()

F32 = mybir.dt.float32
BF16 = mybir.dt.bfloat16
FP8 = mybir.dt.float8e4
BF = ml_dtypes.bfloat16
F8 = (ml_dtypes.float8_e4m3fn if hasattr(ml_dtypes, "float8_e4m3fn")
      else ml_dtypes.float8_e4m3)
AF = mybir.ActivationFunctionType
ALU = mybir.AluOpType

HID = 512
INP = 512
K = 49
L = 49
VOCAB = 10000
B, T = 32, 64
NCORES = 8
BL = B // NCORES          # 4 sequences per core
NBT = BL * T              # 256 (t-major: col = t*BL + b)
G4 = 4 * HID              # 2048
NKC = HID // 128          # 4 k-chunks
NMT = G4 // 128           # 16 gate m-tiles
NCH = 20                  # vocab chunks
CHN = VOCAB // NCH        # 500
YGRP = 5                  # vocab chunks per y-store descriptor

# gate block layout (host perm = [g, i, f, o]); columns of act tiles
GB = slice(0, 16)         # tanh(g)
IB = slice(16, 32)        # sigmoid(i)
FB = slice(32, 48)        # sigmoid(f)
OB = slice(48, 64)        # sigmoid(o)

LAST_RESULT = None        # BassKernelResults of the most recent run (for test.py)


def _bcast_ap(ap2d, count, pos=1):
    """Insert a zero-stride dim of `count` at free position `pos` of a 2-D AP."""
    dims = list(ap2d.ap)
    dims.insert(pos, [0, count])
    return bass.AP(tensor=ap2d.tensor, offset=ap2d.offset, ap=dims)


def _part_bcast(dram_ap, parts):
    """DRAM AP replicated over `parts` partitions (zero-stride partition dim)."""
    return bass.AP(
        tensor=dram_ap.tensor, offset=dram_ap.offset,
        ap=[[0, parts]] + list(dram_ap.ap),
    )


def _kc_view(drm, rows, cols, nkc):
    """DRAM AP for [rows=nkc*128, cols] viewed as [128(part), nkc, cols]."""
    ap = drm[:]
    return bass.AP(tensor=ap.tensor, offset=ap.offset,
                   ap=[[cols, 128], [cols * 128, nkc], [1, cols]])


def _build():
    nc = bass.Bass()

    d_xT = nc.dram_tensor("xT", [INP, NBT], BF16, kind="ExternalInput")
    d_vT = nc.dram_tensor("vT", [HID, BL * L], BF16, kind="ExternalInput")
    d_vnat = nc.dram_tensor("vnat", [BL * L, HID], BF16, kind="ExternalInput")
    d_wihT = nc.dram_tensor("wihT", [INP, G4], BF16, kind="ExternalInput")
    d_whhT = nc.dram_tensor("whhT", [HID, G4], FP8, kind="ExternalInput")
    d_wxsT = nc.dram_tensor("wxsT", [INP, HID], BF16, kind="ExternalInput")
    d_whsT = nc.dram_tensor("whsT", [HID, HID], BF16, kind="ExternalInput")
    d_wgT = nc.dram_tensor("wgT", [HID, K], BF16, kind="ExternalInput")
    d_wsT = nc.dram_tensor("wsT", [HID, K], BF16, kind="ExternalInput")
    d_wvT = nc.dram_tensor("wvT", [HID, K], BF16, kind="ExternalInput")
    d_wmlpT = nc.dram_tensor("wmlpT", [HID, VOCAB], BF16, kind="ExternalInput")
    d_b4 = nc.dram_tensor("b4", [128, NMT], F32, kind="ExternalInput")
    d_wh = nc.dram_tensor("wh", [K], BF16, kind="ExternalInput")
    d_y = nc.dram_tensor("y", [NBT, VOCAB], BF16, kind="ExternalOutput")

    ident = nc.inline_tensor(np.eye(128, dtype=np.float32), name="ident128")
    ident_bf = nc.inline_tensor(np.eye(128, dtype=ml_dtypes.bfloat16),
                                name="ident128bf")

    from contextlib import ExitStack

    with tile.TileContext(nc) as tc, ExitStack() as es:
        consts = es.enter_context(tc.tile_pool(name="consts", bufs=1))
        state = es.enter_context(tc.tile_pool(name="state", bufs=1))
        dram = es.enter_context(tc.tile_pool(name="dram", bufs=1, space="DRAM"))

        # ---- loads: one descriptor per tensor, 2 queues, ACT kept free ---
        def sbuf_kc(name, cols, dt=BF16):
            return consts.tile([128, NKC, cols], dt, tag=name, name=name)

        id_bf = consts.tile([128, 128], BF16, tag="identbf")
        nc.gpsimd.dma_start(out=id_bf[:], in_=ident_bf[:])
        id_sb = consts.tile([128, 128], F32, tag="ident")
        nc.gpsimd.dma_start(out=id_sb[:], in_=ident[:])
        xT = sbuf_kc("xT", NBT)
        nc.sync.dma_start(out=xT[:], in_=_kc_view(d_xT, INP, NBT, NKC))
        wihT = sbuf_kc("wihT", G4)
        # split big weights across both queues (halves of the kc dim)
        nc.gpsimd.dma_start(
            out=wihT[:, 0:2, :],
            in_=bass.AP(tensor=d_wihT[:].tensor, offset=0,
                        ap=[[G4, 128], [G4 * 128, 2], [1, G4]]))
        nc.sync.dma_start(
            out=wihT[:, 2:4, :],
            in_=bass.AP(tensor=d_wihT[:].tensor, offset=2 * 128 * G4,
                        ap=[[G4, 128], [G4 * 128, 2], [1, G4]]))
        whhT = sbuf_kc("whhT", G4, FP8)
        nc.scalar.dma_start(
            out=whhT[:, 0:2, :],
            in_=bass.AP(tensor=d_whhT[:].tensor, offset=0,
                        ap=[[G4, 128], [G4 * 128, 2], [1, G4]]))
        nc.scalar.dma_start(
            out=whhT[:, 2:4, :],
            in_=bass.AP(tensor=d_whhT[:].tensor, offset=2 * 128 * G4,
                        ap=[[G4, 128], [G4 * 128, 2], [1, G4]]))
        b4 = consts.tile([128, NMT], F32, tag="b4")
        nc.sync.dma_start(out=b4[:], in_=d_b4[:])

        wxsT = sbuf_kc("wxsT", HID)
        nc.gpsimd.dma_start(out=wxsT[:], in_=_kc_view(d_wxsT, INP, HID, NKC))
        whsT = sbuf_kc("whsT", HID)
        nc.sync.dma_start(out=whsT[:], in_=_kc_view(d_whsT, HID, HID, NKC))
        wgT = sbuf_kc("wgT", K)
        nc.gpsimd.dma_start(out=wgT[:], in_=_kc_view(d_wgT, HID, K, NKC))
        wsT = sbuf_kc("wsT", K)
        nc.sync.dma_start(out=wsT[:], in_=_kc_view(d_wsT, HID, K, NKC))
        wvT = sbuf_kc("wvT", K)
        nc.gpsimd.dma_start(out=wvT[:], in_=_kc_view(d_wvT, HID, K, NKC))
        vT = sbuf_kc("vT", BL * L)
        nc.sync.dma_start(out=vT[:], in_=_kc_view(d_vT, HID, BL * L, NKC))

        vnat = consts.tile([L, BL, HID], BF16, tag="vnat")
        nc.gpsimd.dma_start(
            out=vnat[:],
            in_=bass.AP(tensor=d_vnat[:].tensor, offset=0,
                        ap=[[HID, L], [HID * L, BL], [1, HID]]))

        wh_rep = consts.tile([128, K], BF16, tag="wh_rep")
        nc.sync.dma_start(out=wh_rep[:], in_=_part_bcast(d_wh[:], 128))
        half1 = consts.tile([1, 128], F32, tag="half1")
        nc.vector.memset(half1[:], 0.5)   # beta broadcast carries the 0.5 of
                                          # the sentinel (tanh+1)/2 folding

        # Wmlp resident in SBUF; the 4 big DMAs are issued from inside the
        # phase-B loop (steps 26..38) so the HBM/SBUF-write traffic misses
        # both the phase-A window and the late-B MLP-hoist window.
        wmlp = consts.tile([128, NKC, VOCAB], BF16, tag="wmlp")

        def _wmlp_load(kc):
            nc.sync.dma_start(out=wmlp[:, kc, :],
                              in_=d_wmlpT[kc * 128 : (kc + 1) * 128, :])

        # ---- persistent state -------------------------------------------
        xg = state.tile([128, T, NMT, BL], BF16, tag="xg")
        xs = state.tile([128, NKC, T, BL], F32, tag="xs")
        tanhc = state.tile([128, NKC, T, BL], F32, tag="tanhc")
        h_bf = state.tile([128, NKC, T + 1, BL], BF16, tag="h_bf")
        c_st = state.tile([128, NKC * BL], F32, tag="c_st")
        vk_rep = state.tile([128, L, K], BF16, tag="vk_rep")
        s_f32 = state.tile([128, NKC, NBT], F32, tag="s_f32")
        s_bf = state.tile([128, NKC, NBT], BF16, tag="s_bf")
        u_bf = state.tile([128, NKC, NBT], BF16, tag="u_bf")
        hg_bf = state.tile([128, 2, K], BF16, tag="hg_bf")
        hg_f = state.tile([128, 2, K], F32, tag="hg_f")
        wh_f = state.tile([128, K], F32, tag="wh_f")
        zt_st = state.tile([128, 2, K], F32, tag="zt_st")
        ez_sb = state.tile([128, 2, K], F32, tag="ez_sb")
        ezs_f = state.tile([128, 2, K], F32, tag="ezs_f")
        ezT = state.tile([L, NBT], BF16, tag="ezT")
        scal = state.tile([128, 2, 8], F32, tag="scal")
        frow = state.tile([1, NBT], F32, tag="frow")
        fbc = state.tile([128, NBT], F32, tag="fbc")
        hh_f = state.tile([128, NKC, 128], F32, tag="hh_f")  # h cast, per-ti
        zin = state.tile([128, L, K], BF16, tag="zin")

        d_vk = dram.tile([BL * L, K], BF16, tag="d_vk")
        d_vkrep = dram.tile([128, L * K], BF16, tag="d_vkrep")

        nc.vector.memset(c_st[:], 0.0)
        nc.vector.memset(h_bf[:, :, 0, :], 0.0)
        nc.vector.tensor_copy(wh_f[:], wh_rep[:])

        # PSUM budget (8 banks): pg blocks 4 + aux 2 + mlp 2
        with tc.tile_pool(name="aux_ps", bufs=2, space="PSUM") as aux_ps, \
             tc.tile_pool(name="pb_psum", bufs=1, space="PSUM") as pb_psum, \
             tc.tile_pool(name="pb_tmp", bufs=3) as pb_tmp, \
             tc.tile_pool(name="mlp_ps", bufs=2, space="PSUM") as mlp_ps, \
             tc.tile_pool(name="pc_tmp", bufs=2) as pc_tmp, \
             tc.tile_pool(name="mlp_out", bufs=3) as mlp_out:

            def _bcast_col(ap_col, n1, n2):
                """[128,1] AP -> [128, n1, n2] zero-stride broadcast."""
                return bass.AP(tensor=ap_col.tensor, offset=ap_col.offset,
                               ap=[list(ap_col.ap[0]), [0, n1], [0, n2]])

            TH = T

            # ============ phase A: x_gates projection =====================
            def _xg_item(mt, th):
                # pg banks are idle until phase B; rotating phase A's GEMM
                # tiles across them (6 buffers total) removes the WAR stalls
                # of a 2-deep pipeline.
                r = mt % 6
                if r < 2:
                    p = aux_ps.tile([128, TH * BL], F32, tag="aux",
                                    name="pa")
                else:
                    p = pb_psum.tile([128, TH * BL], F32, tag=f"pg{r - 2}",
                                     name="pa")
                for kc in range(NKC):
                    nc.tensor.matmul(
                        p[:],
                        lhsT=wihT[:, kc, mt * 128 : (mt + 1) * 128],
                        rhs=xT[:, kc, th * TH * BL : (th + 1) * TH * BL],
                        start=(kc == 0),
                        stop=(kc == NKC - 1),
                    )
                # bias-add + copy to xg on DVE (ACT kept clear for the
                # phase-B chain; first-half copies run pre-B anyway)
                if th == 0 and mt % 2 == 0:
                    nc.scalar.activation(
                        out=xg[:, th * TH : (th + 1) * TH, mt, :],
                        in_=p[:].rearrange("p (t b) -> p t b", b=BL),
                        func=AF.Identity,
                        bias=b4[:, mt : mt + 1],
                        scale=1.0,
                    )
                else:
                    nc.vector.tensor_add(
                        xg[:, th * TH : (th + 1) * TH, mt, :],
                        p[:].rearrange("p (t b) -> p t b", b=BL),
                        _bcast_col(b4[:, mt : mt + 1], TH, BL),
                    )

            # PE warm-up (HAM un-throttle) + ACT sigmoid-table preload while
            # the weight DMAs stream in; id_sb arrives first on SWDGE.
            warm = state.tile([128, 2], F32, tag="warm")
            wps = aux_ps.tile([128, 128], F32, tag="aux")
            for w in range(48):
                nc.tensor.matmul(wps[:], lhsT=id_bf[:], rhs=id_bf[:],
                                 start=(w == 0), stop=(w == 47))
            nc.scalar.activation(out=warm[:, 0:1], in_=wps[:, 0:1],
                                 func=AF.Sigmoid)
            nc.scalar.activation(out=warm[:, 1:2], in_=wps[:, 0:1],
                                 func=AF.Tanh)

            for mt in range(NMT):
                _xg_item(mt, 0)

            # deferred work, one item per early phase-B step
            def _xs_item(st):
                p = aux_ps.tile([128, NBT], F32, tag="aux")
                for kc in range(NKC):
                    nc.tensor.matmul(
                        p[:],
                        lhsT=wxsT[:, kc, st * 128 : (st + 1) * 128],
                        rhs=xT[:, kc, :],
                        start=(kc == 0),
                        stop=(kc == NKC - 1),
                    )
                nc.vector.tensor_copy(
                    xs[:, st, :, :], p[:].rearrange("p (t b) -> p t b", b=BL))

            def _vk_item(half):
                rows = 128 if half == 0 else BL * L - 128
                p = aux_ps.tile([128, K], F32, tag="aux")
                for kc in range(NKC):
                    nc.tensor.matmul(
                        p[:rows, :],
                        lhsT=vT[:, kc, half * 128 : half * 128 + rows],
                        rhs=wvT[:, kc, :],
                        start=(kc == 0),
                        stop=(kc == NKC - 1),
                    )
                tmp = state.tile([128, K], BF16, tag=f"vkh{half}")
                nc.scalar.copy(out=tmp[:rows, :], in_=p[:rows, :])
                nc.gpsimd.dma_start(
                    out=d_vk[half * 128 : half * 128 + rows, :], in_=tmp[:rows, :])

            def _vk_bounce():
                src = d_vk[:]
                dst = d_vkrep[:]
                nc.gpsimd.dma_start(
                    out=bass.AP(tensor=dst.tensor, offset=dst.offset,
                                ap=[[L * K * BL, 128 // BL], [L * K, BL],
                                    [1, L * K]]),
                    in_=bass.AP(tensor=src.tensor, offset=src.offset,
                                ap=[[0, 128 // BL], [L * K, BL], [1, L * K]]),
                )

            def _vk_load():
                nc.gpsimd.dma_start(
                    out=vk_rep[:].rearrange("p l k -> p (l k)"), in_=d_vkrep[:])

            deferred = [lambda st=st: _xs_item(st) for st in range(NKC)]
            deferred += [lambda: _vk_item(0), lambda: _vk_item(1),
                         _vk_bounce, _vk_load]

            # ============== phase C: attention + sentinel =================
            # all tanh/exp (one ACT set); sentinel sigmoid folded into tanh.
            def c_pre_items(ti):
                cols = slice(1 + ti * 32, 1 + ti * 32 + 32)   # current h
                pcols = slice(0 + ti * 32, 0 + ti * 32 + 32)  # h_prev
                bt = slice(ti * 128, (ti + 1) * 128)

                items = []

                def _hg():
                    ph = aux_ps.tile([128, K], F32, tag="aux")
                    for kc in range(NKC):
                        nc.tensor.matmul(
                            ph[:], lhsT=h_bf[:, kc, cols, :], rhs=wgT[:, kc, :],
                            start=(kc == 0), stop=(kc == NKC - 1),
                        )
                    nc.vector.tensor_copy(hg_bf[:, ti, :], ph[:])
                    nc.scalar.copy(out=hg_f[:, ti, :], in_=ph[:])
                items.append(_hg)

                # z-chain in L slices (bf16); muls alternate GP/DVE
                def _z(idx, l0, l1):
                    nc.vector.tensor_add(
                        zin[:, l0:l1, :], vk_rep[:, l0:l1, :],
                        _bcast_ap(hg_bf[:, ti, :], l1 - l0),
                    )
                    nc.scalar.activation(
                        out=zin[:, l0:l1, :], in_=zin[:, l0:l1, :], func=AF.Tanh
                    )
                    nc.vector.tensor_mul(
                        zin[:, l0:l1, :], zin[:, l0:l1, :],
                        _bcast_ap(wh_rep[:], l1 - l0),
                    )
                    nc.vector.tensor_reduce(
                        out=zt_st[:, ti, l0:l1], in_=zin[:, l0:l1, :],
                        axis=mybir.AxisListType.X, op=ALU.add,
                    )
                zsl = ((0, 7), (7, 13), (13, 19), (19, 25),
                       (25, 31), (31, 37), (37, 43), (43, L)) if ti == 0 \
                    else ((0, 25), (25, L))
                for idx, (l0, l1) in enumerate(zsl):
                    items.append(lambda idx=idx, l0=l0, l1=l1: _z(idx, l0, l1))

                # sentinel: 2s = (tanh(x/2)+1) * tanh(c); Ws carries the 0.5
                def _sent(st):
                    ps = aux_ps.tile([128, 128], F32, tag="aux")
                    for kc in range(NKC):
                        nc.tensor.matmul(
                            ps[:], lhsT=whsT[:, kc, st * 128 : (st + 1) * 128],
                            rhs=h_bf[:, kc, pcols, :],
                            start=(kc == 0), stop=(kc == NKC - 1),
                        )
                    pssb = pc_tmp.tile([128, 128], F32, tag="pssb")
                    nc.vector.tensor_add(
                        pssb[:].rearrange("p (t b) -> p t b", b=BL),
                        ps[:].rearrange("p (t b) -> p t b", b=BL),
                        xs[:, st, ti * 32 : (ti + 1) * 32, :],
                    )
                    nc.scalar.activation(out=pssb[:], in_=pssb[:],
                                         func=AF.Tanh, scale=0.5)
                    nc.vector.scalar_tensor_tensor(
                        out=s_f32[:, st, bt].rearrange("p (t b) -> p t b", b=BL),
                        in0=pssb[:].rearrange("p (t b) -> p t b", b=BL),
                        scalar=1.0,
                        in1=tanhc[:, st, ti * 32 : (ti + 1) * 32, :],
                        op0=ALU.add, op1=ALU.mult,
                    )
                    nc.vector.tensor_copy(s_bf[:, st, bt], s_f32[:, st, bt])
                for st in range(NKC):
                    items.append(lambda st=st: _sent(st))

                # sentinel gate pre-activation: tanh((0.5 Ws)@ 2s + hg) . wh
                def _sws():
                    pw = aux_ps.tile([128, K], F32, tag="aux")
                    for kc in range(NKC):
                        nc.tensor.matmul(
                            pw[:], lhsT=s_bf[:, kc, bt], rhs=wsT[:, kc, :],
                            start=(kc == 0), stop=(kc == NKC - 1),
                        )
                    sws = pc_tmp.tile([128, K], F32, tag="sws")
                    nc.vector.tensor_add(sws[:], pw[:], hg_f[:, ti, :])
                    nc.scalar.activation(out=sws[:], in_=sws[:], func=AF.Tanh)
                    nc.vector.tensor_mul(sws[:], sws[:], wh_f[:])
                    nc.vector.tensor_reduce(
                        out=scal[:, ti, 1:2], in_=sws[:],
                        axis=mybir.AxisListType.X, op=ALU.add,
                    )
                items.append(_sws)
                return items

            def c_exp_part(ti):
                bt = slice(ti * 128, (ti + 1) * 128)
                nc.scalar.activation(
                    out=ez_sb[:, ti, :], in_=zt_st[:, ti, :], func=AF.Exp,
                    accum_out=scal[:, ti, 0:1],
                )
                nc.scalar.activation(out=scal[:, ti, 1:2], in_=scal[:, ti, 1:2],
                                     func=AF.Exp)
                nc.vector.tensor_add(scal[:, ti, 2:3], scal[:, ti, 0:1],
                                     scal[:, ti, 1:2])
                nc.vector.reciprocal(scal[:, ti, 3:4], scal[:, ti, 2:3])
                nc.vector.tensor_mul(scal[:, ti, 4:5], scal[:, ti, 1:2],
                                     scal[:, ti, 3:4])
                nc.vector.tensor_scalar_mul(ezs_f[:, ti, :], ez_sb[:, ti, :],
                                            scal[:, ti, 3:4])
                pt = aux_ps.tile([K, 128], F32, tag="aux")
                nc.tensor.transpose(pt[:], ezs_f[:, ti, :], id_sb[:])
                nc.vector.tensor_copy(ezT[:, bt], pt[:])
                pf = aux_ps.tile([1, 128], F32, tag="aux")
                nc.tensor.transpose(pf[:], scal[:, ti, 4:5], id_sb[:])
                nc.vector.tensor_copy(frow[:, bt], pf[:])
                pfb = aux_ps.tile([128, 128], F32, tag="aux")
                # half1 (0.5) carries the sentinel folding into beta
                nc.tensor.matmul(pfb[:], lhsT=half1[:], rhs=frow[:, bt],
                                 start=True, stop=True)
                nc.vector.tensor_copy(fbc[:, bt], pfb[:])

                for hc in range(NKC):
                    nc.vector.tensor_copy(
                        hh_f[:, hc, :],
                        h_bf[:, hc, 1 + ti * 32 : 1 + ti * 32 + 32, :]
                        .rearrange("p t b -> p (t b)"),
                    )
                    pc = aux_ps.tile([128, 128], F32, tag="aux")
                    for b in range(BL):
                        nc.tensor.matmul(
                            pc[:].rearrange("p (t b) -> p t b", b=BL)[:, :, b],
                            lhsT=vnat[:, b, hc * 128 : (hc + 1) * 128],
                            rhs=ezT[:, bt].rearrange("p (t b) -> p t b", b=BL)[:, :, b],
                            start=True, stop=True,
                        )
                    us = pc_tmp.tile([128, 128], F32, tag="us")
                    nc.vector.tensor_mul(us[:], s_f32[:, hc, bt], fbc[:, bt])
                    nc.vector.tensor_add(us[:], us[:], pc[:])
                    nc.vector.tensor_add(us[:], us[:], hh_f[:, hc, :])
                    nc.vector.tensor_copy(u_bf[:, hc, bt], us[:])

            ygrp_state = {}

            def mlp_chunk(ti, nch):
                g0 = (nch // YGRP) * YGRP
                key = (ti, g0)
                if key not in ygrp_state:
                    ygrp_state[key] = mlp_out.tile([128, YGRP * CHN], BF16,
                                                   tag="ysb", name="ysb")
                ysb = ygrp_state[key]
                j = nch - g0
                py = mlp_ps.tile([128, CHN], F32, tag="py")
                for kc in range(NKC):
                    nc.tensor.matmul(
                        py[:],
                        lhsT=u_bf[:, kc, ti * 128 : (ti + 1) * 128],
                        rhs=wmlp[:, kc, nch * CHN : (nch + 1) * CHN],
                        start=(kc == 0), stop=(kc == NKC - 1),
                    )
                if ti == 0:
                    # hoisted into phase B: half-width DVE casts block the
                    # recurrence chain for at most ~0.26us each
                    h = CHN // 2
                    nc.vector.tensor_copy(
                        ysb[:, j * CHN : j * CHN + h], py[:, 0:h])
                    nc.vector.tensor_copy(
                        ysb[:, j * CHN + h : (j + 1) * CHN], py[:, h:CHN])
                elif nch % 2 == 0:
                    nc.vector.tensor_copy(ysb[:, j * CHN : (j + 1) * CHN],
                                          py[:])
                else:
                    nc.scalar.copy(out=ysb[:, j * CHN : (j + 1) * CHN],
                                   in_=py[:])
                if j == YGRP - 1:
                    # all y stores on the SP queue: the SWDGE (gpsimd) ring
                    # then empties long before the tail, so the final
                    # all-engine barrier doesn't wait on its ~4us drain.
                    nc.sync.dma_start(
                        out=d_y[ti * 128 : (ti + 1) * 128,
                                g0 * CHN : (g0 + YGRP) * CHN],
                        in_=ysb[:],
                    )

            def mlp_ti(ti):
                for nch in range(NCH):
                    mlp_chunk(ti, nch)


            # ============== phase B: serial LSTM recurrence ===============
            # burst tile order [g, i, f, o]; chain overlaps the burst tail.
            for t in range(T):
                # recurrence ops outrank hoisted fillers in the scheduler's
                # ready-heap: shift their priorities down by a constant
                # (ordering among them stays unique and monotone)
                tc.cur_priority -= 1000000
                pgs = [pb_psum.tile([128, NKC * BL], F32, tag=f"pg{blk}",
                                    name=f"pg{blk}")
                       for blk in range(4)]
                prev_mm = None
                for blk in range(4):
                    for sub in range(4):
                        mt = blk * 4 + sub
                        for kc in range(NKC):
                            # start=True clears has_written for the WHOLE
                            # bank, so only the block's first matmul may set
                            # it; later sub-tiles first-write via the cleared
                            # has_written bits (overwrite) then accumulate.
                            mm = nc.tensor.matmul(
                                pgs[blk][:, sub * BL : (sub + 1) * BL],
                                lhsT=whhT[:, kc, mt * 128 : (mt + 1) * 128],
                                rhs=h_bf[:, kc, t, :],
                                start=(sub == 0 and kc == 0),
                                stop=False,
                                skip_group_check=True,
                            )
                            if prev_mm is not None:
                                add_dep_helper(mm.ins, prev_mm.ins, sync=False,
                                               reason="psum group order")
                            prev_mm = mm
                    # fold the +xg term into the accumulation with a bf16
                    # identity matmul; the gate activation then reads PSUM
                    # directly (no DVE add on the critical path).
                    mm = nc.tensor.matmul(
                        pgs[blk][:],
                        lhsT=id_bf[:],
                        rhs=xg[:, t, blk * 4 : blk * 4 + 4, :]
                        .rearrange("p m b -> p (m b)"),
                        start=False, stop=True, skip_group_check=True,
                    )
                    add_dep_helper(mm.ins, prev_mm.ins, sync=False,
                                   reason="psum group order")
                    prev_mm = mm

                act = pb_tmp.tile([128, NMT * BL], F32, tag="act")
                act3 = act[:].rearrange("p (m b) -> p m b", b=BL)

                # ACT order: tanh_g, sig_i, sig_f, sig_o, tanh_c (PSUM src)
                # DVE order: cmul, cadd, hmul; ig on GpSimd
                nc.scalar.activation(out=act[:, GB], in_=pgs[0][:],
                                     func=AF.Tanh)
                nc.scalar.activation(out=act[:, IB], in_=pgs[1][:],
                                     func=AF.Sigmoid)
                nc.scalar.activation(out=act[:, FB], in_=pgs[2][:],
                                     func=AF.Sigmoid)

                ig = pb_tmp.tile([128, NKC * BL], F32, tag="ig")
                nc.gpsimd.tensor_mul(ig[:], act[:, IB], act[:, GB])
                nc.vector.tensor_mul(c_st[:], act[:, FB], c_st[:])
                nc.scalar.activation(out=act[:, OB], in_=pgs[3][:],
                                     func=AF.Sigmoid)
                nc.vector.tensor_add(c_st[:], c_st[:], ig[:])
                nc.scalar.activation(
                    out=tanhc[:, :, t, :],
                    in_=c_st[:].rearrange("p (k b) -> p k b", b=BL),
                    func=AF.Tanh,
                )
                nc.vector.tensor_mul(
                    h_bf[:, :, t + 1, :],
                    act3[:, 12:16, :],
                    tanhc[:, :, t, :],
                )

                tc.cur_priority += 1000000
                if t < len(deferred):
                    deferred[t]()
                if t >= 26 and t % 4 == 2 and (t - 26) // 4 < NKC:
                    _wmlp_load((t - 26) // 4)

            for it in c_pre_items(0):
                it()
            c_exp_part(0)
            mlp_ti(0)
            for it in c_pre_items(1):
                it()
            c_exp_part(1)
            mlp_ti(1)

    return nc


_NC_CACHE = None


def kernel(**inputs):
    global _NC_CACHE, LAST_RESULT
    x = np.asarray(inputs["x"], np.float32)
    V = np.asarray(inputs["V"], np.float32)
    Wih = np.asarray(inputs["Wih"], np.float32)
    Whh = np.asarray(inputs["Whh"], np.float32)
    bih = np.asarray(inputs["bih"], np.float32)
    bhh = np.asarray(inputs["bhh"], np.float32)
    Wx_s = np.asarray(inputs["Wx_s"], np.float32)
    Wh_s = np.asarray(inputs["Wh_s"], np.float32)
    Wv = np.asarray(inputs["Wv"], np.float32)
    Wg = np.asarray(inputs["Wg"], np.float32)
    Wh_att = np.asarray(inputs["Wh_att"], np.float32)
    Ws = np.asarray(inputs["Ws"], np.float32)
    Wmlp = np.asarray(inputs["Wmlp"], np.float32)
    bmlp = np.asarray(inputs["bmlp"], np.float32)

    if _NC_CACHE is None:
        _NC_CACHE = _build()
    nc = _NC_CACHE

    # permute gate order [i,f,g,o] -> [g,i,f,o] (see phase B chain schedule)
    perm = np.concatenate([
        np.arange(1024, 1536), np.arange(0, 512),
        np.arange(512, 1024), np.arange(1536, 2048)
    ])
    shared = {
        "wihT": np.ascontiguousarray(Wih.T[:, perm]).astype(BF),
        "whhT": np.ascontiguousarray(Whh.T[:, perm]).astype(F8),
        "wxsT": np.ascontiguousarray(Wx_s.T).astype(BF),
        "whsT": np.ascontiguousarray(Wh_s.T).astype(BF),
        "wgT": np.ascontiguousarray(Wg.T).astype(BF),
        # sentinel sigmoid -> (tanh(x/2)+1)/2: s is computed scaled by 2,
        # compensated by 0.5 here and in the beta broadcast (half1)
        "wsT": np.ascontiguousarray(0.5 * Ws.T).astype(BF),
        "wvT": np.ascontiguousarray(Wv.T).astype(BF),
        "wmlpT": np.ascontiguousarray(Wmlp.T).astype(BF),
        "b4": np.ascontiguousarray((bih + bhh)[perm].reshape(NMT, 128).T),
        "wh": np.ascontiguousarray(Wh_att[0]).astype(BF),
    }
    in_maps = []
    for c in range(NCORES):
        xi = x[c * BL : (c + 1) * BL]          # [BL, T, INP]
        Vi = V[c * BL : (c + 1) * BL]          # [BL, L, HID]
        xT = np.ascontiguousarray(xi.transpose(2, 1, 0).reshape(INP, NBT)).astype(BF)
        vflat = Vi.reshape(BL * L, HID)
        in_maps.append(dict(shared,
                            xT=xT,
                            vT=np.ascontiguousarray(vflat.T).astype(BF),
                            vnat=np.ascontiguousarray(vflat).astype(BF)))

    trace = os.environ.get("BASS_KERNEL_TRACE", "0") == "1"
    res = run_bass_kernel_spmd(nc, in_maps, core_ids=list(range(NCORES)), trace=trace)
    LAST_RESULT = res

    out = np.empty((B, T, VOCAB), np.float32)
    for c in range(NCORES):
        yc = res.results[c]["y"].astype(np.float32).reshape(T, BL, VOCAB)
        out[c * BL : (c + 1) * BL] = yc.transpose(1, 0, 2)
    out += bmlp
    return out
